# revision 41
# baseline (speedup 1.0000x reference)
"""Multi-head attention Trainium2 kernel (B=2, S=4096, D=512, H=8).

Sharding: 8 cores = 2 batches x 4 sequence-chunks of the query. Each core
computes full attention (all 8 heads) for its 1024 query rows against the
full 4096-long key/value sequence of its batch, including the output
projection. Host combine is a pure concat.

Default variant "v4" per-core dataflow (bf16, scores never touch HBM):
  - projections on PE (bf16): kT/qT in [dk, s] layout, v'[s, h*65] natural
    layout with a ones-column per head (softmax-denominator row-sum trick)
  - scoresT[k,q] = kT-slice.T @ qT-slice (dk=64 contraction, head pairs at
    base partitions 0/64), one [128,1024] PSUM group per (k-tile, pair)
  - exp distributed across engines per the V4_CYCLE schedule: 'A' = ACT
    activation-exp; 'S'/'D' = Schraudolph bit-trick (DVE mult-add into an
    int32 whose bits are the f32 exp, then bitcast-copy to bf16 on
    Pool/DVE), +-3%% on the offloaded third of the weights
  - PV in natural orientation: att[q,65] += ep-slice.T @ v'-slice, moving
    dim 65 (vs 512 transposed) -> half the PE time; 4 q-subtile
    accumulators packed per PSUM bank via the lazy pending-zero semantics
  - normalize on DVE (per-partition reciprocal x tensor_scalar -> bf16),
    PE transpose (identity matmul) restores attnT[c, q] for the output
    projection out[q,:] = attnT-slices.T @ Wo.T-tiles
  - software pipelining: scores/exp run LOOKAHEAD=5 groups ahead of PV so
    exp latency never blocks the PE; kT/v' projection interleaves with the
    first attention unit ("chase") to keep ACT busy during the prologue
"""

import sys

sys.path.insert(0, "/opt/trn_rl_repo")
sys.path.insert(0, "/root/.axon_site/_ro/trn_rl_repo")

import numpy as np

B, S, D, H, DK = 2, 4096, 512, 8, 64
NCORES = 8
BSHARD = NCORES // B          # 4 sequence shards per batch
SQ = S // BSHARD              # 1024 query rows per core
NCH = SQ // 512               # 2 q-chunks of 512
NKT = S // 128                # 32 k-tiles
NPAIR = H // 2                # 4 head pairs
VW = DK + 1                   # 65: per-head v width incl. ones column

_cache: dict = {}


def _build(repeat=1, variant="full"):
    ck = (repeat, variant)
    if ck in _cache:
        return _cache[ck]

    import concourse.mybir as mybir
    import concourse.tile as tile
    from concourse import bacc
    from concourse.bass import ts

    F32R = mybir.dt.float32r
    F32 = mybir.dt.float32
    BF16 = mybir.dt.bfloat16
    EXP = mybir.ActivationFunctionType.Exp

    nc = bacc.Bacc("TRN2", target_bir_lowering=False, debug=False,
                   num_devices=NCORES)

    if variant in ("v2a", "v2b"):
        _build_v2(nc, mybir, tile, ts, interleave=(variant == "v2b"),
                  repeat=repeat)
        nc.compile()
        _cache[ck] = nc
        return nc

    if variant == "v3":
        _build_v3(nc, mybir, tile, ts, repeat=repeat)
        nc.compile()
        _cache[ck] = nc
        return nc

    if variant.startswith("v4"):
        _build_v4(nc, mybir, tile, ts, repeat=repeat,
                  cycle=V4_CYCLE, fp8sc=("fp8" in variant),
                  dbg=("dbg" in variant))
        nc.compile()
        _cache[ck] = nc
        return nc

    if variant.startswith("v5"):
        _build_v5(nc, mybir, tile, ts, repeat=repeat, cycle=V4_CYCLE,
                  dbg=("dbg" in variant))
        nc.compile()
        _cache[ck] = nc
        return nc

    xq = nc.dram_tensor("xq_t", [D, SQ], F32R, kind="ExternalInput")
    xk = nc.dram_tensor("xk_t", [D, S], F32R, kind="ExternalInput")
    xv = nc.dram_tensor("xv_t", [D, S], F32R, kind="ExternalInput")
    wq = nc.dram_tensor("wq_t", [D, D], F32R, kind="ExternalInput")
    wk = nc.dram_tensor("wk_t", [D, D], F32R, kind="ExternalInput")
    wv = nc.dram_tensor("wv_t", [D, D], F32R, kind="ExternalInput")
    wo = nc.dram_tensor("wo_t", [D, D], F32R, kind="ExternalInput")
    out = nc.dram_tensor("out", [SQ, D], F32, kind="ExternalOutput")
    if dbg:
        dbg_qt = nc.dram_tensor("dbg_qt", [128, SQ], BF16, kind="ExternalOutput")
        dbg_kt = nc.dram_tensor("dbg_kt", [128, S], BF16, kind="ExternalOutput")
        dbg_vp = nc.dram_tensor("dbg_vp", [128, H * VW], BF16, kind="ExternalOutput")
        dbg_ep = nc.dram_tensor("dbg_ep", [128, 1024], BF16, kind="ExternalOutput")
        dbg_anat = nc.dram_tensor("dbg_anat", [128, 512], BF16, kind="ExternalOutput")
        dbg_att = nc.dram_tensor("dbg_att", [128, 2 * 4 * 72], F32, kind="ExternalOutput")
        dbg_attn = nc.dram_tensor("dbg_attn", [128, 512], BF16, kind="ExternalOutput")

    with tile.TileContext(nc) as tc:
      for _rep in range(repeat):
        with (
            tc.tile_pool(name="kt", bufs=4) as kt_pool,
            tc.tile_pool(name="qt", bufs=4) as qt_pool,
            tc.tile_pool(name="vp", bufs=32) as vp_pool,
            tc.tile_pool(name="attn", bufs=4) as attn_pool,
            tc.tile_pool(name="wop", bufs=4) as wo_pool,
            tc.tile_pool(name="ep", bufs=7) as ep_pool,
            tc.tile_pool(name="sm", bufs=2) as sm_pool,
            tc.tile_pool(name="ob", bufs=2) as ob_pool,
        ):
            kt = [kt_pool.tile([128, S], F32R, tag="kt", name=f"kt{i}") for i in range(4)]
            qt = [qt_pool.tile([128, SQ], F32R, tag="qt", name=f"qt{i}") for i in range(4)]
            vp = [vp_pool.tile([128, H * VW], BF16, tag="vp", name=f"vp{i}")
                  for i in range(NKT)]
            attn = [attn_pool.tile([128, SQ], F32R, tag="attn", name=f"attn{i}")
                    for i in range(4)]
            if variant in ("proj_only", "nopv"):
                for i in range(4):
                    nc.gpsimd.memset(attn[i][:].bitcast(F32), 0.0)
            wot = [wo_pool.tile([128, D], F32R, tag="wo", name=f"wot{i}") for i in range(4)]
            for d in range(4):
                nc.sync.dma_start(wot[d][:], wo[ts(d, 128), :])

            # ---------------- projections ----------------
            with (
                tc.tile_pool(name="wtmp", bufs=4) as w_pool,
                tc.tile_pool(name="xs", bufs=8) as xs_pool,
                tc.tile_pool(name="pp", bufs=4, space="PSUM") as pp_pool,
            ):
                # kT[dk, s] = sum_d Wk.T[d, dk] * xkT[d, s]
                wkt = [w_pool.tile([128, D], F32R, tag="w", name=f"w{i}") for i in range(4)]
                for d in range(4):
                    nc.sync.dma_start(wkt[d][:], wk[ts(d, 128), :])
                for ch in range(S // 512):
                    xst = [xs_pool.tile([128, 512], F32R, tag="xs", name=f"xs{i}")
                           for i in range(4)]
                    for d in range(4):
                        nc.sync.dma_start(
                            xst[d][:], xk[ts(d, 128), ts(ch, 512)])
                    for m in range(4):
                        ps = pp_pool.tile([128, 512], F32, tag="pp")
                        for d in range(4):
                            nc.tensor.matmul(
                                ps[:], wkt[d][:, ts(m, 128)], xst[d][:],
                                start=(d == 0), stop=(d == 3))
                        nc.scalar.copy(kt[m][:, ts(ch, 512)], ps[:])

                # qT[dk, q] likewise from the core's query slice
                wqt = [w_pool.tile([128, D], F32R, tag="w", name=f"w{i}") for i in range(4)]
                for d in range(4):
                    nc.sync.dma_start(wqt[d][:], wq[ts(d, 128), :])
                for ch in range(NCH):
                    xst = [xs_pool.tile([128, 512], F32R, tag="xs", name=f"xs{i}")
                           for i in range(4)]
                    for d in range(4):
                        nc.sync.dma_start(
                            xst[d][:], xq[ts(d, 128), ts(ch, 512)])
                    for m in range(4):
                        ps = pp_pool.tile([128, 512], F32, tag="pp")
                        for d in range(4):
                            nc.tensor.matmul(
                                ps[:], wqt[d][:, ts(m, 128)], xst[d][:],
                                start=(d == 0), stop=(d == 3))
                        nc.scalar.copy(qt[m][:, ts(ch, 512)], ps[:])

                # v'[s, h*65] = xvT-slices.T @ Wv.T, plus ones columns
                wvt = [w_pool.tile([128, D], F32R, tag="w", name=f"w{i}") for i in range(4)]
                for d in range(4):
                    nc.sync.dma_start(wvt[d][:], wv[ts(d, 128), :])
                for sc in range(S // 512):
                    xst = [xs_pool.tile([128, 512], F32R, tag="xs", name=f"xs{i}")
                           for i in range(4)]
                    for d in range(4):
                        nc.sync.dma_start(
                            xst[d][:], xv[ts(d, 128), ts(sc, 512)])
                    for st in range(4):
                        k_idx = sc * 4 + st
                        ps = pp_pool.tile([128, 512], F32, tag="pp")
                        for d in range(4):
                            nc.tensor.matmul(
                                ps[:], xst[d][:, ts(st, 128)], wvt[d][:],
                                start=(d == 0), stop=(d == 3))
                        v3 = vp[k_idx][:].rearrange(
                            "p (h c) -> p h c", c=VW)
                        nc.gpsimd.memset(v3[:, :, DK:VW], 1.0)
                        nc.vector.tensor_copy(
                            v3[:, :, 0:DK],
                            ps[:].rearrange("p (h c) -> p h c", c=DK))

            # ---------------- attention + output projection ----------------
            with (
                tc.tile_pool(name="sc", bufs=3, space="PSUM") as sc_pool,
                tc.tile_pool(name="acc", bufs=2, space="PSUM") as acc_pool,
            ):
                for ch in range(NCH):
                    for p in range(NPAIR if variant != "proj_only" else 0):
                        att = [acc_pool.tile([VW, 512], F32, tag="acc", name=f"att{i}")
                               for i in range(2)]
                        # (head, ktile) sequence; heads of the pair adjacent
                        seq = [(2 * p + hh, k) for k in range(NKT)
                               for hh in range(2)]
                        for g in range(0, len(seq), 3):
                            items = seq[g:g + 3]
                            n = len(items)
                            scps = sc_pool.tile([128, 1536], F32, tag="sc")
                            for slot, (h, k) in enumerate(items):
                                off = (h % 2) * 64
                                nc.tensor.matmul(
                                    scps[:, ts(slot, 512)],
                                    kt[p][off:off + 64, ts(k, 128)],
                                    qt[p][off:off + 64, ts(ch, 512)],
                                    start=True, stop=True)
                            ep = ep_pool.tile([128, 1536], BF16, tag="ep")
                            if variant == "dve_exp":
                                nc.vector.tensor_copy(
                                    ep[:, 0:n * 512], scps[:, 0:n * 512])
                            else:
                                nc.scalar.activation(
                                    ep[:, 0:n * 512], scps[:, 0:n * 512],
                                    EXP, scale=0.125)
                            if variant != "nopv":
                                for slot, (h, k) in enumerate(items):
                                    nc.tensor.matmul(
                                        att[h % 2][:],
                                        vp[k][:, h * VW:(h + 1) * VW],
                                        ep[:, ts(slot, 512)],
                                        start=(k == 0), stop=(k == NKT - 1))
                        for hh in range(2 if variant != "nopv" else 0):
                            rc = sm_pool.tile([1, 512], F32, tag="rc")
                            nc.vector.reciprocal(rc[:], att[hh][DK:VW, :])
                            rep = sm_pool.tile([64, 512], F32, tag="rep")
                            nc.gpsimd.partition_broadcast(rep[:], rc[:])
                            nc.vector.tensor_mul(
                                attn[p][hh * 64:(hh + 1) * 64, ts(ch, 512)],
                                att[hh][0:DK, :], rep[:])
                    # output projection for this chunk of 512 q rows
                    for sbi in range(4):
                        po = acc_pool.tile([128, 512], F32, tag="acc")
                        for ct in range(4):
                            nc.tensor.matmul(
                                po[:],
                                attn[ct][:, ch * 512 + sbi * 128:
                                         ch * 512 + (sbi + 1) * 128],
                                wot[ct][:],
                                start=(ct == 0), stop=(ct == 3))
                        oo = ob_pool.tile([128, 512], F32, tag="ob")
                        nc.vector.tensor_copy(oo[:], po[:])
                        nc.sync.dma_start(
                            out[ch * 512 + sbi * 128:
                                ch * 512 + (sbi + 1) * 128, :], oo[:])

    nc.compile()
    _cache[ck] = nc
    return nc




def _build_v2(nc, mybir, tile, ts, interleave, repeat=1):
    """v2: bf16 inputs/weights for projections, [128,1024] DMA chunks, all
    weights loaded upfront, proj psums carved from the score-psum pool, and
    (v2b) unit-(0,0) attention interleaved with the kT/v' projections."""
    F32R = mybir.dt.float32r
    F32 = mybir.dt.float32
    BF16 = mybir.dt.bfloat16
    EXP = mybir.ActivationFunctionType.Exp

    xq = nc.dram_tensor("xq_t", [D, SQ], BF16, kind="ExternalInput")
    xk = nc.dram_tensor("xk_t", [D, S], BF16, kind="ExternalInput")
    xv = nc.dram_tensor("xv_t", [D, S], BF16, kind="ExternalInput")
    wq = nc.dram_tensor("wq_t", [D, D], BF16, kind="ExternalInput")
    wk = nc.dram_tensor("wk_t", [D, D], BF16, kind="ExternalInput")
    wv = nc.dram_tensor("wv_t", [D, D], BF16, kind="ExternalInput")
    wo = nc.dram_tensor("wo_t", [D, D], F32R, kind="ExternalInput")
    out = nc.dram_tensor("out", [SQ, D], F32, kind="ExternalOutput")
    if dbg:
        dbg_qt = nc.dram_tensor("dbg_qt", [128, SQ], BF16, kind="ExternalOutput")
        dbg_kt = nc.dram_tensor("dbg_kt", [128, S], BF16, kind="ExternalOutput")
        dbg_vp = nc.dram_tensor("dbg_vp", [128, H * VW], BF16, kind="ExternalOutput")
        dbg_ep = nc.dram_tensor("dbg_ep", [128, 1024], BF16, kind="ExternalOutput")
        dbg_anat = nc.dram_tensor("dbg_anat", [128, 512], BF16, kind="ExternalOutput")
        dbg_att = nc.dram_tensor("dbg_att", [128, 2 * 4 * 72], F32, kind="ExternalOutput")
        dbg_attn = nc.dram_tensor("dbg_attn", [128, 512], BF16, kind="ExternalOutput")

    with tile.TileContext(nc) as tc:
      for _rep in range(repeat):
        with (
            tc.tile_pool(name="kt", bufs=4) as kt_pool,
            tc.tile_pool(name="qt", bufs=4) as qt_pool,
            tc.tile_pool(name="vp", bufs=32) as vp_pool,
            tc.tile_pool(name="attn", bufs=4) as attn_pool,
            tc.tile_pool(name="wop", bufs=4) as wo_pool,
            tc.tile_pool(name="wb", bufs=12) as w_pool,
            tc.tile_pool(name="xs", bufs=8) as xs_pool,
            tc.tile_pool(name="ep", bufs=7) as ep_pool,
            tc.tile_pool(name="sm", bufs=2) as sm_pool,
            tc.tile_pool(name="ob", bufs=2) as ob_pool,
            tc.tile_pool(name="sc", bufs=3, space="PSUM") as sc_pool,
            tc.tile_pool(name="acc", bufs=2, space="PSUM") as acc_pool,
        ):
            kt = [kt_pool.tile([128, S], F32R, tag="kt", name=f"kt{i}")
                  for i in range(4)]
            qt = [qt_pool.tile([128, SQ], F32R, tag="qt", name=f"qt{i}")
                  for i in range(4)]
            vp = [vp_pool.tile([128, H * VW], BF16, tag="vp", name=f"vp{i}")
                  for i in range(NKT)]
            attn = [attn_pool.tile([128, SQ], F32R, tag="attn",
                                   name=f"attn{i}") for i in range(4)]
            wot = [wo_pool.tile([128, D], F32R, tag="wo", name=f"wot{i}")
                   for i in range(4)]
            wqt = [w_pool.tile([128, D], BF16, tag="wb", name=f"wq{i}")
                   for i in range(4)]
            wkt = [w_pool.tile([128, D], BF16, tag="wb", name=f"wk{i}")
                   for i in range(4)]
            wvt = [w_pool.tile([128, D], BF16, tag="wb", name=f"wv{i}")
                   for i in range(4)]
            for d in range(4):
                nc.sync.dma_start(wqt[d][:], wq[ts(d, 128), :])
                nc.scalar.dma_start(wot[d][:], wo[ts(d, 128), :])
                nc.sync.dma_start(wkt[d][:], wk[ts(d, 128), :])
                nc.scalar.dma_start(wvt[d][:], wv[ts(d, 128), :])

            # rotating [128,512] psum slices carved from whole sc-pool tiles
            state = {"t": None, "i": 3, "n": 0}

            def psum512():
                if state["i"] == 3:
                    state["t"] = sc_pool.tile(
                        [128, 1536], F32, tag="sc",
                        name=f"pj{state['n']}")
                    state["n"] += 1
                    state["i"] = 0
                sl = state["t"][:, ts(state["i"], 512)]
                state["i"] += 1
                return sl

            # ---- qT projection (1 super-chunk of 1024 q cols) ----
            xst = [xs_pool.tile([128, 1024], BF16, tag="xs", name=f"xq{i}")
                   for i in range(4)]
            for d in range(4):
                nc.sync.dma_start(xst[d][:], xq[ts(d, 128), :])
            for m in range(4):
                for sub in range(2):
                    ps = psum512()
                    for d in range(4):
                        nc.tensor.matmul(
                            ps, wqt[d][:, ts(m, 128)],
                            xst[d][:, ts(sub, 512)],
                            start=(d == 0), stop=(d == 3))
                    nc.scalar.copy(qt[m][:, ts(sub, 512)], ps)

            # ---- interleaved kT / v' projections + unit (0,0) ----
            att0 = [acc_pool.tile([VW, 512], F32, tag="acc",
                                  name=f"att0_{i}") for i in range(2)]
            seq0 = [(hh, k) for k in range(NKT) for hh in range(2)]
            gptr = [0]

            def emit_unit0_groups(gmax):
                while gptr[0] < gmax:
                    g = gptr[0]
                    items = seq0[3 * g:3 * g + 3]
                    if not items:
                        break
                    n = len(items)
                    scps = sc_pool.tile([128, 1536], F32, tag="sc",
                                        name=f"sc0_{g}")
                    for slot, (hh, k) in enumerate(items):
                        off = hh * 64
                        nc.tensor.matmul(
                            scps[:, ts(slot, 512)],
                            kt[0][off:off + 64, ts(k, 128)],
                            qt[0][off:off + 64, 0:512],
                            start=True, stop=True)
                    ep = ep_pool.tile([128, 1536], BF16, tag="ep",
                                      name=f"ep0_{g}")
                    nc.scalar.activation(ep[:, 0:n * 512],
                                         scps[:, 0:n * 512],
                                         EXP, scale=0.125)
                    for slot, (hh, k) in enumerate(items):
                        nc.tensor.matmul(
                            att0[hh][:], vp[k][:, hh * VW:(hh + 1) * VW],
                            ep[:, ts(slot, 512)],
                            start=(k == 0), stop=(k == NKT - 1))
                    gptr[0] += 1

            NGROUPS = (len(seq0) + 2) // 3
            for j in range(4):  # super-chunks of 1024 s cols
                xst = [xs_pool.tile([128, 1024], BF16, tag="xs",
                                    name=f"xk{j}_{i}") for i in range(4)]
                for d in range(4):
                    nc.sync.dma_start(
                        xst[d][:], xk[ts(d, 128), ts(j, 1024)])
                for m in range(4):
                    for sub in range(2):
                        ps = psum512()
                        for d in range(4):
                            nc.tensor.matmul(
                                ps, wkt[d][:, ts(m, 128)],
                                xst[d][:, ts(sub, 512)],
                                start=(d == 0), stop=(d == 3))
                        nc.scalar.copy(
                            kt[m][:, j * 1024 + sub * 512:
                                  j * 1024 + (sub + 1) * 512], ps)
                xvt = [xs_pool.tile([128, 1024], BF16, tag="xs",
                                    name=f"xv{j}_{i}") for i in range(4)]
                for d in range(4):
                    nc.scalar.dma_start(
                        xvt[d][:], xv[ts(d, 128), ts(j, 1024)])
                for st in range(8):
                    k_idx = j * 8 + st
                    ps = psum512()
                    for d in range(4):
                        nc.tensor.matmul(
                            ps, xvt[d][:, ts(st, 128)], wvt[d][:],
                            start=(d == 0), stop=(d == 3))
                    v3 = vp[k_idx][:].rearrange("p (h c) -> p h c", c=VW)
                    nc.gpsimd.memset(v3[:, :, DK:VW], 1.0)
                    nc.vector.tensor_copy(
                        v3[:, :, 0:DK],
                        ps.rearrange("p (h c) -> p h c", c=DK))
                if interleave:
                    emit_unit0_groups((16 * (j + 1)) // 3 if j < 3
                                      else NGROUPS)
            if not interleave:
                emit_unit0_groups(NGROUPS)

            def finish_unit(att, p, ch):
                for hh in range(2):
                    rc = sm_pool.tile([1, 512], F32, tag="rc")
                    nc.vector.reciprocal(rc[:], att[hh][DK:VW, :])
                    rep = sm_pool.tile([64, 512], F32, tag="rep")
                    nc.gpsimd.partition_broadcast(rep[:], rc[:])
                    nc.vector.tensor_mul(
                        attn[p][hh * 64:(hh + 1) * 64, ts(ch, 512)],
                        att[hh][0:DK, :], rep[:])

            def emit_unit(p, ch):
                att = [acc_pool.tile([VW, 512], F32, tag="acc",
                                     name=f"att{p}{ch}_{i}")
                       for i in range(2)]
                seq = [(2 * p + hh, k) for k in range(NKT)
                       for hh in range(2)]
                for g in range(0, len(seq), 3):
                    items = seq[g:g + 3]
                    n = len(items)
                    scps = sc_pool.tile([128, 1536], F32, tag="sc",
                                        name=f"sc{p}{ch}_{g}")
                    for slot, (h, k) in enumerate(items):
                        off = (h % 2) * 64
                        nc.tensor.matmul(
                            scps[:, ts(slot, 512)],
                            kt[p][off:off + 64, ts(k, 128)],
                            qt[p][off:off + 64, ts(ch, 512)],
                            start=True, stop=True)
                    ep = ep_pool.tile([128, 1536], BF16, tag="ep",
                                      name=f"ep{p}{ch}_{g}")
                    nc.scalar.activation(ep[:, 0:n * 512],
                                         scps[:, 0:n * 512],
                                         EXP, scale=0.125)
                    for slot, (h, k) in enumerate(items):
                        nc.tensor.matmul(
                            att[h % 2][:],
                            vp[k][:, h * VW:(h + 1) * VW],
                            ep[:, ts(slot, 512)],
                            start=(k == 0), stop=(k == NKT - 1))
                finish_unit(att, p, ch)

            def emit_outproj(ch):
                for sbi in range(4):
                    po = acc_pool.tile([128, 512], F32, tag="acc",
                                       name=f"po{ch}{sbi}")
                    for ct in range(4):
                        nc.tensor.matmul(
                            po[:],
                            attn[ct][:, ch * 512 + sbi * 128:
                                     ch * 512 + (sbi + 1) * 128],
                            wot[ct][:],
                            start=(ct == 0), stop=(ct == 3))
                    oo = ob_pool.tile([128, 512], F32, tag="ob",
                                      name=f"oo{ch}{sbi}")
                    nc.vector.tensor_copy(oo[:], po[:])
                    nc.sync.dma_start(
                        out[ch * 512 + sbi * 128:
                            ch * 512 + (sbi + 1) * 128, :], oo[:])

            finish_unit(att0, 0, 0)
            for p in (1, 2, 3):
                emit_unit(p, 0)
            emit_outproj(0)
            for p in (0, 1, 2, 3):
                emit_unit(p, 1)
            emit_outproj(1)


def _build_v3(nc, mybir, tile, ts, repeat=1):
    """v3: like v2b but q-chunks of 256 so one [65,512] PSUM bank holds both
    heads' accumulators of a unit -> two attention units in flight; kT/qT
    psum evacuation moved off ACT to DVE."""
    F32R = mybir.dt.float32r
    F32 = mybir.dt.float32
    BF16 = mybir.dt.bfloat16
    EXP = mybir.ActivationFunctionType.Exp
    QC = 256
    NC_CH = SQ // QC          # 4 q-chunks per core
    GRP = 6                   # 256-wide slices per exp group (N=1536)

    xq = nc.dram_tensor("xq_t", [D, SQ], BF16, kind="ExternalInput")
    xk = nc.dram_tensor("xk_t", [D, S], BF16, kind="ExternalInput")
    xv = nc.dram_tensor("xv_t", [D, S], BF16, kind="ExternalInput")
    wq = nc.dram_tensor("wq_t", [D, D], BF16, kind="ExternalInput")
    wk = nc.dram_tensor("wk_t", [D, D], BF16, kind="ExternalInput")
    wv = nc.dram_tensor("wv_t", [D, D], BF16, kind="ExternalInput")
    wo = nc.dram_tensor("wo_t", [D, D], F32R, kind="ExternalInput")
    out = nc.dram_tensor("out", [SQ, D], F32, kind="ExternalOutput")
    if dbg:
        dbg_qt = nc.dram_tensor("dbg_qt", [128, SQ], BF16, kind="ExternalOutput")
        dbg_kt = nc.dram_tensor("dbg_kt", [128, S], BF16, kind="ExternalOutput")
        dbg_vp = nc.dram_tensor("dbg_vp", [128, H * VW], BF16, kind="ExternalOutput")
        dbg_ep = nc.dram_tensor("dbg_ep", [128, 1024], BF16, kind="ExternalOutput")
        dbg_anat = nc.dram_tensor("dbg_anat", [128, 512], BF16, kind="ExternalOutput")
        dbg_att = nc.dram_tensor("dbg_att", [128, 2 * 4 * 72], F32, kind="ExternalOutput")
        dbg_attn = nc.dram_tensor("dbg_attn", [128, 512], BF16, kind="ExternalOutput")

    with tile.TileContext(nc) as tc:
      for _rep in range(repeat):
        with (
            tc.tile_pool(name="kt", bufs=4) as kt_pool,
            tc.tile_pool(name="qt", bufs=4) as qt_pool,
            tc.tile_pool(name="vp", bufs=32) as vp_pool,
            tc.tile_pool(name="attn", bufs=4) as attn_pool,
            tc.tile_pool(name="wop", bufs=4) as wo_pool,
            tc.tile_pool(name="wb", bufs=12) as w_pool,
            tc.tile_pool(name="xs", bufs=8) as xs_pool,
            tc.tile_pool(name="ep", bufs=7) as ep_pool,
            tc.tile_pool(name="sm", bufs=4) as sm_pool,
            tc.tile_pool(name="ob", bufs=2) as ob_pool,
            tc.tile_pool(name="sc", bufs=3, space="PSUM") as sc_pool,
            tc.tile_pool(name="acc", bufs=2, space="PSUM") as acc_pool,
        ):
            kt = [kt_pool.tile([128, S], F32R, tag="kt", name=f"kt{i}")
                  for i in range(4)]
            qt = [qt_pool.tile([128, SQ], F32R, tag="qt", name=f"qt{i}")
                  for i in range(4)]
            vp = [vp_pool.tile([128, H * VW], BF16, tag="vp", name=f"vp{i}")
                  for i in range(NKT)]
            attn = [attn_pool.tile([128, SQ], F32R, tag="attn",
                                   name=f"attn{i}") for i in range(4)]
            wot = [wo_pool.tile([128, D], F32R, tag="wo", name=f"wot{i}")
                   for i in range(4)]
            wqt = [w_pool.tile([128, D], BF16, tag="wb", name=f"wq{i}")
                   for i in range(4)]
            wkt = [w_pool.tile([128, D], BF16, tag="wb", name=f"wk{i}")
                   for i in range(4)]
            wvt = [w_pool.tile([128, D], BF16, tag="wb", name=f"wv{i}")
                   for i in range(4)]
            for d in range(4):
                nc.sync.dma_start(wqt[d][:], wq[ts(d, 128), :])
                nc.scalar.dma_start(wot[d][:], wo[ts(d, 128), :])
                nc.sync.dma_start(wkt[d][:], wk[ts(d, 128), :])
                nc.scalar.dma_start(wvt[d][:], wv[ts(d, 128), :])

            state = {"t": None, "i": 3, "n": 0}

            def psum512():
                if state["i"] == 3:
                    state["t"] = sc_pool.tile(
                        [128, 1536], F32, tag="sc", name=f"pj{state['n']}")
                    state["n"] += 1
                    state["i"] = 0
                sl = state["t"][:, ts(state["i"], 512)]
                state["i"] += 1
                return sl

            # ---- qT projection ----
            xst = [xs_pool.tile([128, 1024], BF16, tag="xs", name=f"xq{i}")
                   for i in range(4)]
            for d in range(4):
                nc.sync.dma_start(xst[d][:], xq[ts(d, 128), :])
            for m in range(4):
                for sub in range(2):
                    ps = psum512()
                    for d in range(4):
                        nc.tensor.matmul(
                            ps, wqt[d][:, ts(m, 128)],
                            xst[d][:, ts(sub, 512)],
                            start=(d == 0), stop=(d == 3))
                    nc.vector.tensor_copy(qt[m][:, ts(sub, 512)], ps)

            # ---- attention unit machinery ----
            def unit_groups(p, c, att, gfrom, gto):
                seq = [(hh, k) for k in range(NKT) for hh in range(2)]
                for g in range(gfrom, gto):
                    items = seq[GRP * g:GRP * g + GRP]
                    if not items:
                        return
                    n = len(items)
                    scps = sc_pool.tile([128, 1536], F32, tag="sc",
                                        name=f"sc{p}_{c}_{g}")
                    for slot, (hh, k) in enumerate(items):
                        off = hh * 64
                        nc.tensor.matmul(
                            scps[:, ts(slot, QC)],
                            kt[p][off:off + 64, ts(k, 128)],
                            qt[p][off:off + 64, ts(c, QC)],
                            start=True, stop=True)
                    ep = ep_pool.tile([128, 1536], BF16, tag="ep",
                                      name=f"ep{p}_{c}_{g}")
                    nc.scalar.activation(ep[:, 0:n * QC], scps[:, 0:n * QC],
                                         EXP, scale=0.125)
                    for slot, (hh, k) in enumerate(items):
                        nc.tensor.matmul(
                            att[:, ts(hh, QC)],
                            vp[k][:, (2 * p + hh) * VW:
                                  (2 * p + hh + 1) * VW],
                            ep[:, ts(slot, QC)],
                            start=(k == 0 and hh == 0),
                            stop=(k == NKT - 1 and hh == 1),
                            skip_group_check=True)

            NG = (2 * NKT + GRP - 1) // GRP  # 11 groups per unit

            def finish_unit(att, p, c):
                for hh in range(2):
                    rc = sm_pool.tile([1, QC], F32, tag="rc")
                    nc.vector.reciprocal(rc[:], att[DK:VW, ts(hh, QC)])
                    rep = sm_pool.tile([64, QC], F32, tag="rep")
                    nc.gpsimd.partition_broadcast(rep[:], rc[:])
                    nc.vector.tensor_mul(
                        attn[p][hh * 64:(hh + 1) * 64, ts(c, QC)],
                        att[0:DK, ts(hh, QC)], rep[:])

            def emit_unit(p, c):
                att = acc_pool.tile([VW, 512], F32, tag="acc",
                                    name=f"att{p}_{c}")
                unit_groups(p, c, att, 0, NG)
                finish_unit(att, p, c)

            # ---- interleaved kT / v' projections + units (0,0),(0,1) ----
            attA = acc_pool.tile([VW, 512], F32, tag="acc", name="attA")
            attB = acc_pool.tile([VW, 512], F32, tag="acc", name="attB")
            gA, gB = [0], [0]

            def chase(j):
                lim = min(NG, (16 * (j + 1)) // GRP) if j < 3 else NG
                unit_groups(0, 0, attA, gA[0], lim)
                gA[0] = max(gA[0], lim)
                unit_groups(0, 1, attB, gB[0], lim)
                gB[0] = max(gB[0], lim)

            for j in range(4):
                xst = [xs_pool.tile([128, 1024], BF16, tag="xs",
                                    name=f"xk{j}_{i}") for i in range(4)]
                for d in range(4):
                    nc.sync.dma_start(
                        xst[d][:], xk[ts(d, 128), ts(j, 1024)])
                for m in range(4):
                    for sub in range(2):
                        ps = psum512()
                        for d in range(4):
                            nc.tensor.matmul(
                                ps, wkt[d][:, ts(m, 128)],
                                xst[d][:, ts(sub, 512)],
                                start=(d == 0), stop=(d == 3))
                        nc.vector.tensor_copy(
                            kt[m][:, j * 1024 + sub * 512:
                                  j * 1024 + (sub + 1) * 512], ps)
                xvt = [xs_pool.tile([128, 1024], BF16, tag="xs",
                                    name=f"xv{j}_{i}") for i in range(4)]
                for d in range(4):
                    nc.scalar.dma_start(
                        xvt[d][:], xv[ts(d, 128), ts(j, 1024)])
                for st in range(8):
                    k_idx = j * 8 + st
                    ps = psum512()
                    for d in range(4):
                        nc.tensor.matmul(
                            ps, xvt[d][:, ts(st, 128)], wvt[d][:],
                            start=(d == 0), stop=(d == 3))
                    v3_ = vp[k_idx][:].rearrange("p (h c) -> p h c", c=VW)
                    nc.gpsimd.memset(v3_[:, :, DK:VW], 1.0)
                    nc.vector.tensor_copy(
                        v3_[:, :, 0:DK],
                        ps.rearrange("p (h c) -> p h c", c=DK))
                chase(j)
            finish_unit(attA, 0, 0)
            finish_unit(attB, 0, 1)

            def emit_outproj(ch):
                for sbi in range(4):
                    po = acc_pool.tile([128, 512], F32, tag="acc",
                                       name=f"po{ch}{sbi}")
                    for ct in range(4):
                        nc.tensor.matmul(
                            po[:],
                            attn[ct][:, ch * 512 + sbi * 128:
                                     ch * 512 + (sbi + 1) * 128],
                            wot[ct][:],
                            start=(ct == 0), stop=(ct == 3))
                    oo = ob_pool.tile([128, 512], F32, tag="ob",
                                      name=f"oo{ch}{sbi}")
                    nc.vector.tensor_copy(oo[:], po[:])
                    nc.sync.dma_start(
                        out[ch * 512 + sbi * 128:
                            ch * 512 + (sbi + 1) * 128, :], oo[:])

            for p in (1, 2, 3):
                emit_unit(p, 0)
                emit_unit(p, 1)
            emit_outproj(0)
            for p in (0, 1, 2, 3):
                emit_unit(p, 2)
                emit_unit(p, 3)
            emit_outproj(1)


V4_CYCLE = "AAS"   # exp-group engine schedule: A=ACT, D=DVE, S=DVE+Pool split


def _build_v4(nc, mybir, tile, ts, repeat=1, cycle="AAS", fp8sc=False,
              dbg=False):
    """v4: PV in natural orientation (att[q, dk] accumulates with moving
    dim 65 -> half the PE time of transposed PV), PE transposes to restore
    the [c, q] layout the output projection needs, exp distributed over
    ACT / DVE(+Pool) via a Schraudolph bit-trick on the offloaded groups,
    all projections + scores in bf16, kT/qT/v' psum evacuation on Pool."""
    import math

    from concourse.masks import make_identity

    F32 = mybir.dt.float32
    BF16 = mybir.dt.bfloat16
    I32 = mybir.dt.int32
    EXP = mybir.ActivationFunctionType.Exp
    MULT = mybir.AluOpType.mult
    ADD = mybir.AluOpType.add

    FP8 = mybir.dt.float8e4
    DR = mybir.MatmulPerfMode.DoubleRow
    AW = 72                      # per-subtile stride in att psum (64+den+pad)
    SCALE = 0.0625 if fp8sc else 0.125
    A_CONST = float(SCALE * math.log2(math.e) * (1 << 23))
    B_CONST = float((127 - 0.0575) * (1 << 23))

    xq = nc.dram_tensor("xq_t", [D, SQ], BF16, kind="ExternalInput")
    xk = nc.dram_tensor("xk_t", [D, S], BF16, kind="ExternalInput")
    xv = nc.dram_tensor("xv_t", [D, S], BF16, kind="ExternalInput")
    wq = nc.dram_tensor("wq_t", [D, D], BF16, kind="ExternalInput")
    wk = nc.dram_tensor("wk_t", [D, D], BF16, kind="ExternalInput")
    wv = nc.dram_tensor("wv_t", [D, D], BF16, kind="ExternalInput")
    wo = nc.dram_tensor("wo_t", [D, D], BF16, kind="ExternalInput")
    out = nc.dram_tensor("out", [SQ, D], F32, kind="ExternalOutput")
    if dbg:
        dbg_qt = nc.dram_tensor("dbg_qt", [128, SQ], BF16, kind="ExternalOutput")
        dbg_kt = nc.dram_tensor("dbg_kt", [128, S], BF16, kind="ExternalOutput")
        dbg_vp = nc.dram_tensor("dbg_vp", [128, H * VW], BF16, kind="ExternalOutput")
        dbg_ep = nc.dram_tensor("dbg_ep", [128, 1024], BF16, kind="ExternalOutput")
        dbg_anat = nc.dram_tensor("dbg_anat", [128, 512], BF16, kind="ExternalOutput")
        dbg_att = nc.dram_tensor("dbg_att", [128, 2 * 4 * 72], F32, kind="ExternalOutput")
        dbg_attn = nc.dram_tensor("dbg_attn", [128, 512], BF16, kind="ExternalOutput")

    with tile.TileContext(nc) as tc:
      for _rep in range(repeat):
        with (
            tc.tile_pool(name="kt", bufs=4) as kt_pool,
            tc.tile_pool(name="qt", bufs=4) as qt_pool,
            tc.tile_pool(name="vp", bufs=32) as vp_pool,
            tc.tile_pool(name="attn", bufs=4) as attn_pool,
            tc.tile_pool(name="wb", bufs=16) as w_pool,
            tc.tile_pool(name="xs", bufs=8) as xs_pool,
            tc.tile_pool(name="ep", bufs=7) as ep_pool,
            tc.tile_pool(name="i32", bufs=4) as i32_pool,
            tc.tile_pool(name="sm", bufs=6) as sm_pool,
            tc.tile_pool(name="ob", bufs=2) as ob_pool,
            tc.tile_pool(name="idp", bufs=1) as id_pool,
            tc.tile_pool(name="sc", bufs=2, space="PSUM") as sc_pool,
            tc.tile_pool(name="scs", bufs=1, space="PSUM") as scs_pool,
        ):
            QKDT = FP8 if fp8sc else BF16
            kt = [kt_pool.tile([128, S], QKDT, tag="kt", name=f"kt{i}")
                  for i in range(4)]
            qt = [qt_pool.tile([128, SQ], QKDT, tag="qt", name=f"qt{i}")
                  for i in range(4)]
            vp = [vp_pool.tile([128, H * VW], BF16, tag="vp", name=f"vp{i}")
                  for i in range(NKT)]
            attn_sb = [attn_pool.tile([128, SQ], BF16, tag="attn",
                                      name=f"attn{i}") for i in range(4)]
            wqt = [w_pool.tile([128, D], BF16, tag="wb", name=f"wq{i}")
                   for i in range(4)]
            wkt = [w_pool.tile([128, D], BF16, tag="wb", name=f"wk{i}")
                   for i in range(4)]
            wvt = [w_pool.tile([128, D], BF16, tag="wb", name=f"wv{i}")
                   for i in range(4)]
            wot = [w_pool.tile([128, D], BF16, tag="wb", name=f"wo{i}")
                   for i in range(4)]
            for d in range(4):
                nc.sync.dma_start(wqt[d][:], wq[ts(d, 128), :])
                nc.gpsimd.dma_start(wot[d][:], wo[ts(d, 128), :])
                nc.gpsimd.dma_start(wvt[d][:], wv[ts(d, 128), :])
            ident = id_pool.tile([128, 128], BF16, tag="id", name="ident")
            make_identity(nc, ident[:])

            # rotating [128,512] proj-psum slices carved from sc-pool tiles
            state = {"t": None, "i": 2, "n": 0}

            def psum512():
                if state["i"] == 2:
                    state["t"] = sc_pool.tile(
                        [128, 1024], F32, tag="sc", name=f"pj{state['n']}")
                    state["n"] += 1
                    state["i"] = 0
                sl = state["t"][:, ts(state["i"], 512)]
                state["i"] += 1
                return sl

            gctr = [0]

            def emit_exp(scps, ep_t, force_act=False):
                eng = "A" if force_act else cycle[gctr[0] % len(cycle)]
                gctr[0] += 1
                if eng == "A":
                    nc.scalar.activation(ep_t[:], scps[:], EXP, scale=SCALE)
                    return
                it = i32_pool.tile([128, 1024], I32, tag="i32")
                nc.vector.tensor_scalar(
                    out=it[:], in0=scps[:], scalar1=A_CONST,
                    scalar2=B_CONST, op0=MULT, op1=ADD)
                src = it[:].bitcast(F32)
                if eng == "D":
                    nc.vector.tensor_copy(ep_t[:], src)
                else:
                    nc.gpsimd.tensor_copy(ep_t[:], src)

            def emit_pv(p, att01, ep_t, k, hhs=(0, 1)):
                # k==0, j==0 uses start=True: it marks the whole psum bank
                # pending-zero, so each region's first write overwrites and
                # later writes accumulate -- no separate memset needed.
                for hh in hhs:
                    h = 2 * p + hh
                    for j in range(4):
                        nc.tensor.matmul(
                            att01[hh][:, j * AW:j * AW + VW],
                            ep_t[:, hh * 512 + j * 128:
                                 hh * 512 + (j + 1) * 128],
                            vp[k][:, h * VW:(h + 1) * VW],
                            start=(k == 0 and j == 0), stop=(k == NKT - 1),
                            skip_group_check=True)

            # pend[unit]: up to LOOKAHEAD groups whose scores+exp are
            # emitted but whose PV is not, so the PE program never waits for
            # the exp of a group it just scored (exp latency is hidden by
            # the next groups' score matmuls).
            LOOKAHEAD = 5
            pend = {}

            def unit_groups(p, c, att01, kfrom, kto, force_act=False):
                q = pend.setdefault((p, c), [])
                for k in range(kfrom, kto):
                    tail_act = k >= NKT - 2
                    eng_next = ("A" if (force_act or tail_act)
                                else cycle[gctr[0] % len(cycle)])
                    pool_k = sc_pool if eng_next == "A" else scs_pool
                    scps = pool_k.tile([128, 1024], F32, tag="sc",
                                       name=f"sc{p}_{c}_{k}")
                    for hh in range(2):
                        off = hh * 64
                        if fp8sc:
                            ktv = kt[p][off:off + 64, ts(k, 128)].rearrange(
                                "p (o s) -> p o s",
                                o=1).broadcast_to([64, 2, 128])
                            qtv = qt[p][off:off + 64, ts(c, 512)].rearrange(
                                "p (o s) -> p o s",
                                o=1).broadcast_to([64, 2, 512])
                            nc.tensor.matmul(
                                scps[:, ts(hh, 512)], ktv, qtv,
                                start=True, stop=True, perf_mode=DR)
                        else:
                            nc.tensor.matmul(
                                scps[:, ts(hh, 512)],
                                kt[p][off:off + 64, ts(k, 128)],
                                qt[p][off:off + 64, ts(c, 512)],
                                start=True, stop=True)
                    ep_t = ep_pool.tile([128, 1024], BF16, tag="ep",
                                        name=f"ep{p}_{c}_{k}")
                    emit_exp(scps, ep_t, force_act=force_act or tail_act)
                    if dbg and (p, c, k) == (1, 0, 0):
                        nc.sync.dma_start(dbg_ep[:, :], ep_t[:])
                    q.append((ep_t, k))
                    if len(q) > LOOKAHEAD:
                        pep, pk = q.pop(0)
                        emit_pv(p, att01, pep, pk)

            def normalize_hh(att01, anat, rc, hh):
                attv = att01[hh][:].rearrange("p (j w) -> p j w", w=AW)
                nc.vector.reciprocal(
                    rc[:, hh * 4:(hh + 1) * 4], attv[:, :, DK])
                for j in range(4):
                    nc.vector.tensor_scalar(
                        out=anat[:, hh * 256 + j * 64:
                                 hh * 256 + (j + 1) * 64],
                        in0=att01[hh][:, j * AW:j * AW + DK],
                        scalar1=rc[:, hh * 4 + j:hh * 4 + j + 1],
                        scalar2=None, op0=MULT)

            def flush_unit(p, c, att01, uname):
                # finish head 0's PV first and normalize it on DVE while
                # head 1's PV matmuls still run on the PE
                pends = pend.pop((p, c), [])
                anat = sm_pool.tile([128, 512], BF16, tag="anat",
                                    name=f"anat{uname}")
                rc = sm_pool.tile([128, 8], F32, tag="rc", name=f"rc{uname}")
                for pep, pk in pends:
                    emit_pv(p, att01, pep, pk, hhs=(0,))
                normalize_hh(att01, anat, rc, 0)
                for pep, pk in pends:
                    emit_pv(p, att01, pep, pk, hhs=(1,))
                normalize_hh(att01, anat, rc, 1)
                return anat

            def transpose_store(p, c, anat, pool):
                tp = pool.tile([128, 512], BF16, tag="sc",
                               name=f"tp{p}_{c}")
                for hh in range(2):
                    for j in range(4):
                        nc.tensor.transpose(
                            tp[hh * 64:(hh + 1) * 64, ts(j, 128)],
                            anat[:, hh * 256 + j * 64:
                                 hh * 256 + (j + 1) * 64],
                            ident[:])
                nc.vector.tensor_copy(attn_sb[p][:, ts(c, 512)], tp[:])

            def outproj(ch, pool):
                for sbi in range(4):
                    po = pool.tile([128, 512], F32, tag="sc",
                                        name=f"po{ch}{sbi}")
                    for ct in range(4):
                        nc.tensor.matmul(
                            po[:],
                            attn_sb[ct][:, ch * 512 + sbi * 128:
                                        ch * 512 + (sbi + 1) * 128],
                            wot[ct][:],
                            start=(ct == 0), stop=(ct == 3))
                    oo = ob_pool.tile([128, 512], F32, tag="ob",
                                      name=f"oo{ch}{sbi}")
                    nc.vector.tensor_copy(oo[:], po[:])
                    nc.sync.dma_start(
                        out[ch * 512 + sbi * 128:
                            ch * 512 + (sbi + 1) * 128, :], oo[:])

            # ================= phase 1: projections + chase =================
            with tc.tile_pool(name="attc", bufs=2, space="PSUM") as attc_pool:
                attA = [attc_pool.tile([128, 4 * AW], F32, tag="acc",
                                       name=f"attA{i}") for i in range(2)]

                # qT projection
                xst = [xs_pool.tile([128, 1024], BF16, tag="xs",
                                    name=f"xq{i}") for i in range(4)]
                for d in range(4):
                    nc.sync.dma_start(xst[d][:], xq[ts(d, 128), :])
                for d in range(4):
                    nc.sync.dma_start(wkt[d][:], wk[ts(d, 128), :])
                for m in range(4):
                    for sub in range(2):
                        ps = psum512()
                        for d in range(4):
                            nc.tensor.matmul(
                                ps, wqt[d][:, ts(m, 128)],
                                xst[d][:, ts(sub, 512)],
                                start=(d == 0), stop=(d == 3))
                        nc.scalar.copy(qt[m][:, ts(sub, 512)], ps)

                # kT / v' projections interleaved with units (0,0), (0,1)
                for j in range(4):
                    xst = [xs_pool.tile([128, 1024], BF16, tag="xs",
                                        name=f"xk{j}_{i}") for i in range(4)]
                    for d in range(4):
                        nc.sync.dma_start(
                            xst[d][:], xk[ts(d, 128), ts(j, 1024)])
                    for m in range(4):
                        for sub in range(2):
                            ps = psum512()
                            for d in range(4):
                                nc.tensor.matmul(
                                    ps, wkt[d][:, ts(m, 128)],
                                    xst[d][:, ts(sub, 512)],
                                    start=(d == 0), stop=(d == 3))
                            nc.scalar.copy(
                                kt[m][:, j * 1024 + sub * 512:
                                      j * 1024 + (sub + 1) * 512], ps)
                    xvt = [xs_pool.tile([128, 1024], BF16, tag="xs",
                                        name=f"xv{j}_{i}") for i in range(4)]
                    for d in range(4):
                        nc.gpsimd.dma_start(
                            xvt[d][:], xv[ts(d, 128), ts(j, 1024)])
                    for st in range(8):
                        k_idx = j * 8 + st
                        ps = psum512()
                        for d in range(4):
                            nc.tensor.matmul(
                                ps, xvt[d][:, ts(st, 128)], wvt[d][:],
                                start=(d == 0), stop=(d == 3))
                        v3_ = vp[k_idx][:].rearrange("p (h c) -> p h c", c=VW)
                        nc.gpsimd.memset(v3_[:, :, DK:VW], 1.0)
                        nc.vector.tensor_copy(
                            v3_[:, :, 0:DK],
                            ps.rearrange("p (h c) -> p h c", c=DK))
                    unit_groups(0, 0, attA, 8 * j, 8 * j + 8, force_act=True)
                anatA = flush_unit(0, 0, attA, "A")

            # ================= phase 2: remaining units =================
            with (
                tc.tile_pool(name="att2", bufs=2, space="PSUM") as att2_pool,
            ):
                transpose_store(0, 0, anatA, sc_pool)

                def emit_unit(p, c):
                    att01 = [att2_pool.tile([128, 4 * AW], F32, tag="acc",
                                            name=f"att{p}_{c}_{i}")
                             for i in range(2)]
                    unit_groups(p, c, att01, 0, NKT)
                    anat = flush_unit(p, c, att01, f"{p}_{c}")
                    if dbg and (p, c) == (1, 0):
                        stg = sm_pool.tile([128, 576], F32, tag="dbgs",
                                           name="dbgstg")
                        nc.vector.memset(stg[:], 0.0)
                        for _hh in range(2):
                            for _j in range(4):
                                nc.vector.tensor_copy(
                                    stg[:, _hh * 288 + _j * 72:
                                        _hh * 288 + _j * 72 + VW],
                                    att01[_hh][:, _j * AW:_j * AW + VW])
                        nc.sync.dma_start(dbg_att[:, :], stg[:])
                    if dbg and (p, c) == (1, 0):
                        nc.sync.dma_start(dbg_anat[:, :], anat[:])
                    transpose_store(p, c, anat, sc_pool)
                    if dbg and (p, c) == (1, 0):
                        nc.sync.dma_start(dbg_attn[:, :],
                                          attn_sb[p][:, ts(c, 512)])

                for p in (1, 2, 3):
                    emit_unit(p, 0)
                outproj(0, sc_pool)
                emit_unit(0, 1)
                for p in (1, 2, 3):
                    emit_unit(p, 1)
                outproj(1, sc_pool)
                if dbg:
                    nc.sync.dma_start(dbg_qt[:, :], qt[1][:])
                    nc.sync.dma_start(dbg_kt[:, :], kt[1][:])
                    nc.sync.dma_start(dbg_vp[:, :], vp[0][:])


def _build_v5(nc, mybir, tile, ts, repeat=1, cycle="AAS", dbg=False):
    """v5: like v4 but head-sharded: each core owns 2 heads (one pair) over
    the full 4096-query sequence of its batch. Q/K/V projections shrink 4x
    (only this pair's 128 output dims); the output projection contributes a
    partial product that the host sums across the 4 pair-cores of a batch."""
    import math

    from concourse.masks import make_identity

    F32 = mybir.dt.float32
    BF16 = mybir.dt.bfloat16
    I32 = mybir.dt.int32
    EXP = mybir.ActivationFunctionType.Exp
    MULT = mybir.AluOpType.mult
    ADD = mybir.AluOpType.add

    AW = 72
    SCALE = 0.125
    A_CONST = float(SCALE * math.log2(math.e) * (1 << 23))
    B_CONST = float((127 - 0.0575) * (1 << 23))
    NCH5 = S // 512              # 8 query chunks per core
    PVW = 2 * VW                 # 130: pair width in v-prime

    xq = nc.dram_tensor("xq_t", [D, S], BF16, kind="ExternalInput")
    xk = nc.dram_tensor("xk_t", [D, S], BF16, kind="ExternalInput")
    xv = nc.dram_tensor("xv_t", [D, S], BF16, kind="ExternalInput")
    wq = nc.dram_tensor("wq_t", [D, 128], BF16, kind="ExternalInput")
    wk = nc.dram_tensor("wk_t", [D, 128], BF16, kind="ExternalInput")
    wv = nc.dram_tensor("wv_t", [D, 128], BF16, kind="ExternalInput")
    wo = nc.dram_tensor("wo_t", [128, D], BF16, kind="ExternalInput")
    out = nc.dram_tensor("out", [S, D], F32, kind="ExternalOutput")
    if dbg:
        dbg_qt = nc.dram_tensor("dbg_qt", [128, S], BF16, kind="ExternalOutput")
        dbg_kt = nc.dram_tensor("dbg_kt", [128, S], BF16, kind="ExternalOutput")
        dbg_vp = nc.dram_tensor("dbg_vp", [128, 2 * PVW], BF16, kind="ExternalOutput")
        dbg_ep = nc.dram_tensor("dbg_ep", [128, 1024], BF16, kind="ExternalOutput")
        dbg_attn = nc.dram_tensor("dbg_attn", [128, S], BF16, kind="ExternalOutput")

    with tile.TileContext(nc) as tc:
      for _rep in range(repeat):
        with (
            tc.tile_pool(name="kt", bufs=1) as kt_pool,
            tc.tile_pool(name="qt", bufs=1) as qt_pool,
            tc.tile_pool(name="vp", bufs=32) as vp_pool,
            tc.tile_pool(name="attn", bufs=1) as attn_pool,
            tc.tile_pool(name="wb", bufs=13) as w_pool,
            tc.tile_pool(name="xs", bufs=12) as xs_pool,
            tc.tile_pool(name="ep", bufs=7) as ep_pool,
            tc.tile_pool(name="i32", bufs=4) as i32_pool,
            tc.tile_pool(name="sm", bufs=6) as sm_pool,
            tc.tile_pool(name="ob", bufs=2) as ob_pool,
            tc.tile_pool(name="idp", bufs=1) as id_pool,
            tc.tile_pool(name="sc", bufs=3, space="PSUM") as sc_pool,
        ):
            kt1 = kt_pool.tile([128, S], BF16, tag="kt", name="kt1")
            qt1 = qt_pool.tile([128, S], BF16, tag="qt", name="qt1")
            vp = [vp_pool.tile([128, PVW], BF16, tag="vp", name=f"vp{i}")
                  for i in range(NKT)]
            attn1 = attn_pool.tile([128, S], BF16, tag="attn", name="attn1")
            wqt = [w_pool.tile([128, 128], BF16, tag="wb", name=f"wq{i}")
                   for i in range(4)]
            wkt = [w_pool.tile([128, 128], BF16, tag="wb", name=f"wk{i}")
                   for i in range(4)]
            wvt = [w_pool.tile([128, 128], BF16, tag="wb", name=f"wv{i}")
                   for i in range(4)]
            wot1 = w_pool.tile([128, D], BF16, tag="wb", name="wo1")
            for d in range(4):
                nc.sync.dma_start(wqt[d][:], wq[ts(d, 128), :])
                nc.sync.dma_start(wkt[d][:], wk[ts(d, 128), :])
                nc.gpsimd.dma_start(wvt[d][:], wv[ts(d, 128), :])
            nc.gpsimd.dma_start(wot1[:], wo[:, :])
            ident = id_pool.tile([128, 128], BF16, tag="id", name="ident")
            make_identity(nc, ident[:])

            state = {"t": None, "i": 2, "n": 0}

            def psum512():
                if state["i"] == 2:
                    state["t"] = sc_pool.tile(
                        [128, 1024], F32, tag="sc", name=f"pj{state['n']}")
                    state["n"] += 1
                    state["i"] = 0
                sl = state["t"][:, ts(state["i"], 512)]
                state["i"] += 1
                return sl

            gctr = [0]

            def emit_exp(scps, ep_t, force_act=False):
                eng = "A" if force_act else cycle[gctr[0] % len(cycle)]
                gctr[0] += 1
                if eng == "A":
                    nc.scalar.activation(ep_t[:], scps[:], EXP, scale=SCALE)
                    return
                it = i32_pool.tile([128, 1024], I32, tag="i32")
                nc.vector.tensor_scalar(
                    out=it[:], in0=scps[:], scalar1=A_CONST,
                    scalar2=B_CONST, op0=MULT, op1=ADD)
                src = it[:].bitcast(F32)
                if eng == "D":
                    nc.vector.tensor_copy(ep_t[:], src)
                else:
                    nc.gpsimd.tensor_copy(ep_t[:], src)

            def emit_pv(att01, ep_t, k):
                for hh in range(2):
                    for j in range(4):
                        nc.tensor.matmul(
                            att01[hh][:, j * AW:j * AW + VW],
                            ep_t[:, hh * 512 + j * 128:
                                 hh * 512 + (j + 1) * 128],
                            vp[k][:, hh * VW:(hh + 1) * VW],
                            start=(k == 0 and j == 0), stop=(k == NKT - 1),
                            skip_group_check=True)

            LOOKAHEAD = 5
            pend = {}

            def unit_groups(c, att01, kfrom, kto, force_act=False):
                q = pend.setdefault(c, [])
                for k in range(kfrom, kto):
                    scps = sc_pool.tile([128, 1024], F32, tag="sc",
                                        name=f"sc{c}_{k}")
                    for hh in range(2):
                        off = hh * 64
                        nc.tensor.matmul(
                            scps[:, ts(hh, 512)],
                            kt1[off:off + 64, ts(k, 128)],
                            qt1[off:off + 64, ts(c, 512)],
                            start=True, stop=True)
                    ep_t = ep_pool.tile([128, 1024], BF16, tag="ep",
                                        name=f"ep{c}_{k}")
                    emit_exp(scps, ep_t, force_act=force_act)
                    if dbg and (c, k) == (1, 0):
                        nc.sync.dma_start(dbg_ep[:, :], ep_t[:])
                    q.append((ep_t, k))
                    if len(q) > LOOKAHEAD:
                        pep, pk = q.pop(0)
                        emit_pv(att01, pep, pk)

            def flush_unit(c, att01):
                for pep, pk in pend.pop(c, []):
                    emit_pv(att01, pep, pk)

            def normalize(att01, uname):
                anat = sm_pool.tile([128, 512], BF16, tag="anat",
                                    name=f"anat{uname}")
                rc = sm_pool.tile([128, 8], F32, tag="rc", name=f"rc{uname}")
                for hh in range(2):
                    attv = att01[hh][:].rearrange("p (j w) -> p j w", w=AW)
                    nc.vector.reciprocal(
                        rc[:, hh * 4:(hh + 1) * 4], attv[:, :, DK])
                    for j in range(4):
                        nc.vector.tensor_scalar(
                            out=anat[:, hh * 256 + j * 64:
                                     hh * 256 + (j + 1) * 64],
                            in0=att01[hh][:, j * AW:j * AW + DK],
                            scalar1=rc[:, hh * 4 + j:hh * 4 + j + 1],
                            scalar2=None, op0=MULT)
                return anat

            def transpose_store(c, anat, pool):
                tp = pool.tile([128, 512], BF16, tag="sc", name=f"tp{c}")
                for hh in range(2):
                    for j in range(4):
                        nc.tensor.transpose(
                            tp[hh * 64:(hh + 1) * 64, ts(j, 128)],
                            anat[:, hh * 256 + j * 64:
                                 hh * 256 + (j + 1) * 64],
                            ident[:])
                nc.vector.tensor_copy(attn1[:, ts(c, 512)], tp[:])

            def outproj(ch, pool):
                for sbi in range(4):
                    po = pool.tile([128, 512], F32, tag="sc",
                                   name=f"po{ch}{sbi}")
                    nc.tensor.matmul(
                        po[:],
                        attn1[:, ch * 512 + sbi * 128:
                              ch * 512 + (sbi + 1) * 128],
                        wot1[:], start=True, stop=True)
                    oo = ob_pool.tile([128, 512], F32, tag="ob",
                                      name=f"oo{ch}{sbi}")
                    nc.vector.tensor_copy(oo[:], po[:])
                    nc.sync.dma_start(
                        out[ch * 512 + sbi * 128:
                            ch * 512 + (sbi + 1) * 128, :], oo[:])

            # ============ phase 1: projections + chase of unit 0 ============
            with tc.tile_pool(name="attc", bufs=2, space="PSUM") as attc_pool:
                attA = [attc_pool.tile([128, 4 * AW], F32, tag="acc",
                                       name=f"attA{i}") for i in range(2)]
                for j in range(4):
                    xqt = [xs_pool.tile([128, 1024], BF16, tag="xs",
                                        name=f"xq{j}_{i}") for i in range(4)]
                    for d in range(4):
                        nc.sync.dma_start(
                            xqt[d][:], xq[ts(d, 128), ts(j, 1024)])
                    for sub in range(2):
                        ps = psum512()
                        for d in range(4):
                            nc.tensor.matmul(
                                ps, wqt[d][:], xqt[d][:, ts(sub, 512)],
                                start=(d == 0), stop=(d == 3))
                        nc.scalar.copy(
                            qt1[:, j * 1024 + sub * 512:
                                j * 1024 + (sub + 1) * 512], ps)
                    xkt = [xs_pool.tile([128, 1024], BF16, tag="xs",
                                        name=f"xk{j}_{i}") for i in range(4)]
                    for d in range(4):
                        nc.sync.dma_start(
                            xkt[d][:], xk[ts(d, 128), ts(j, 1024)])
                    for sub in range(2):
                        ps = psum512()
                        for d in range(4):
                            nc.tensor.matmul(
                                ps, wkt[d][:], xkt[d][:, ts(sub, 512)],
                                start=(d == 0), stop=(d == 3))
                        nc.scalar.copy(
                            kt1[:, j * 1024 + sub * 512:
                                j * 1024 + (sub + 1) * 512], ps)
                    xvt = [xs_pool.tile([128, 1024], BF16, tag="xs",
                                        name=f"xv{j}_{i}") for i in range(4)]
                    for d in range(4):
                        nc.gpsimd.dma_start(
                            xvt[d][:], xv[ts(d, 128), ts(j, 1024)])
                    vsl = [None]
                    for st in range(8):
                        k_idx = j * 8 + st
                        if st % 4 == 0:
                            vsl[0] = psum512()
                        ps = vsl[0][:, (st % 4) * 128:(st % 4 + 1) * 128]
                        for d in range(4):
                            nc.tensor.matmul(
                                ps, xvt[d][:, ts(st, 128)], wvt[d][:],
                                start=(d == 0), stop=(d == 3))
                        v3_ = vp[k_idx][:].rearrange("p (h c) -> p h c", c=VW)
                        nc.gpsimd.memset(v3_[:, :, DK:VW], 1.0)
                        nc.vector.tensor_copy(
                            v3_[:, :, 0:DK],
                            ps.rearrange("p (h c) -> p h c", c=DK))
                    unit_groups(0, attA, 8 * j, 8 * j + 8, force_act=True)
                flush_unit(0, attA)
                anatA = normalize(attA, "A")

            # ================= phase 2: remaining units =================
            with (
                tc.tile_pool(name="att2", bufs=2, space="PSUM") as att2_pool,
            ):
                transpose_store(0, anatA, sc_pool)
                outproj(0, sc_pool)

                for c in range(1, NCH5):
                    att01 = [att2_pool.tile([128, 4 * AW], F32, tag="acc",
                                            name=f"att{c}_{i}")
                             for i in range(2)]
                    unit_groups(c, att01, 0, NKT)
                    flush_unit(c, att01)
                    anat = normalize(att01, f"u{c}")
                    transpose_store(c, anat, sc_pool)
                    outproj(c, sc_pool)
                if dbg:
                    nc.sync.dma_start(dbg_qt[:, :], qt1[:])
                    nc.sync.dma_start(dbg_kt[:, :], kt1[:])
                    nc.sync.dma_start(dbg_vp[:, 0:PVW], vp[0][:])
                    nc.sync.dma_start(dbg_vp[:, PVW:2 * PVW], vp[31][:])
                    nc.sync.dma_start(dbg_attn[:, :], attn1[:])


DEFAULT_VARIANT = "v4"


def _fp8_col_perm():
    # m-tile m holds [4 heads] x [half-dk 32] with head-major partitions:
    # m = 2*g + half -> heads 4g..4g+3, dk rows 32*half..32*half+31
    perm = []
    for m in range(4):
        g, half = m // 2, m % 2
        for hi in range(4):
            h = 4 * g + hi
            perm.extend(64 * h + 32 * half + r for r in range(32))
    return np.asarray(perm)


def make_in_maps(query, key, value, Wq, Wk, Wv, Wo, variant):
    import ml_dtypes
    v4 = variant.startswith(("v4", "v5"))
    fp8 = v4 and "fp8" in variant
    xdt = (ml_dtypes.bfloat16 if variant in ("v2a", "v2b", "v3") or v4
           else np.float32)
    query = np.asarray(query, dtype=np.float32)
    key = np.asarray(key, dtype=np.float32)
    value = np.asarray(value, dtype=np.float32)
    w_maps = {
        "wq_t": np.ascontiguousarray(np.asarray(Wq, dtype=np.float32).T).astype(xdt),
        "wk_t": np.ascontiguousarray(np.asarray(Wk, dtype=np.float32).T).astype(xdt),
        "wv_t": np.ascontiguousarray(np.asarray(Wv, dtype=np.float32).T).astype(xdt),
        "wo_t": np.ascontiguousarray(np.asarray(Wo, dtype=np.float32).T).astype(
            xdt if v4 else np.float32),
    }
    if variant.startswith("v5"):
        in_maps = []
        for b in range(B):
            xq_t = np.ascontiguousarray(query[b].T).astype(xdt)
            xk_t = np.ascontiguousarray(key[b].T).astype(xdt)
            xv_t = np.ascontiguousarray(value[b].T).astype(xdt)
            for p0 in range(4):
                cs = slice(128 * p0, 128 * (p0 + 1))
                in_maps.append({
                    "xq_t": xq_t, "xk_t": xk_t, "xv_t": xv_t,
                    "wq_t": np.ascontiguousarray(w_maps["wq_t"][:, cs]),
                    "wk_t": np.ascontiguousarray(w_maps["wk_t"][:, cs]),
                    "wv_t": np.ascontiguousarray(w_maps["wv_t"][:, cs]),
                    "wo_t": np.ascontiguousarray(w_maps["wo_t"][cs, :]),
                })
        return in_maps
    in_maps = []
    for c in range(NCORES):
        b, sh = divmod(c, BSHARD)
        qs = sh * SQ
        xq_t = np.ascontiguousarray(query[b].T[:, qs:qs + SQ]).astype(xdt)
        xk_t = np.ascontiguousarray(key[b].T).astype(xdt)
        xv_t = np.ascontiguousarray(value[b].T).astype(xdt)
        in_maps.append({"xq_t": xq_t, "xk_t": xk_t, "xv_t": xv_t, **w_maps})
    return in_maps


def kernel(query, key, value, Wq, Wk, Wv, Wo, _trace=False, _trace_cores=None):
    from concourse.bass_utils import run_bass_kernel_spmd

    nc = _build(1, DEFAULT_VARIANT)
    in_maps = make_in_maps(query, key, value, Wq, Wk, Wv, Wo, DEFAULT_VARIANT)

    res = run_bass_kernel_spmd(
        nc, in_maps, core_ids=list(range(NCORES)),
        trace=_trace, trace_cores=_trace_cores)
    kernel.last_results = res

    full = np.empty((B, S, D), dtype=np.float32)
    if DEFAULT_VARIANT.startswith("v5"):
        for b in range(B):
            acc = res.results[4 * b]["out"].copy()
            for p0 in range(1, 4):
                acc += res.results[4 * b + p0]["out"]
            full[b] = acc
    else:
        for c in range(NCORES):
            b, sh = divmod(c, BSHARD)
            qs = sh * SQ
            full[b, qs:qs + SQ] = res.results[c]["out"]
    return full

